# revision 32
# baseline (speedup 1.0000x reference)
"""Trainium2 Bass kernel for nn_DecoderCoords_45904610460123.

Strategy: data-parallel over batch (B=8) across 8 NeuronCores; each core runs
the full 6-layer transformer decoder for one batch element. Activations are
kept feature-major in SBUF ("xT" layout: [128 part, 4 dtile, 1024 seq]); all
matmuls run in float32r (FP22, full PE rate at free-dim>=256). Attention uses
a transposed-scores layout (keys on partitions) so softmax denominators come
from an augmented-V matmul row and no transposes are needed in the inner loop.
Block-causal masking is applied additively on PSUM only on diagonal tiles;
fully-masked regions are skipped entirely (both in the score matmuls and in
the probability@V accumulation, via column-sliced PSUM accumulation).
"""

import numpy as np

import concourse.bass as bass
import concourse.mybir as mybir
import concourse.tile as tile
from concourse import bacc
from concourse.bass_utils import run_bass_kernel_spmd
from concourse.masks import make_identity

# Force walrus/bacc to satisfy all our activation funcs (Exp, Ln, Square,
# Copy/Identity) from the single natural_log_exp_and_others table set, so the
# kernel pays exactly one ACT_TABLE_LOAD instead of toggling between sets.
_TARGET_SET = "natural_log_exp_and_others"
_orig_get_act_tables = bacc.get_activation_tables


def _patched_get_act_tables(module_arch):
    tabs = dict(_orig_get_act_tables(module_arch))
    keep = tabs[_TARGET_SET]
    out = {}
    for name, funcs in tabs.items():
        if name == _TARGET_SET:
            out[name] = funcs
        else:
            out[name] = funcs - keep
    return out


bacc.get_activation_tables = _patched_get_act_tables

P = 128
S = 1024          # T*A
D = 512
DT = D // P       # 4
NQ = 512
NI = S // NQ      # 2
KT = S // P       # 8
H = 8
HD = 64
F = 2048
FT = F // P       # 16
L = 6
EPS = 1e-5
B, T, A = 8, 64, 16

F32 = mybir.dt.float32
F32R = mybir.dt.float32r
AX = mybir.AluOpType
AF = mybir.ActivationFunctionType


def sl(i, sz):
    return slice(i * sz, (i + 1) * sz)


def build_nc():
    nc = bacc.Bacc("TRN2", target_bir_lowering=False, debug=False)

    dr = {}

    def din(name, shape, dt=F32R):
        dr[name] = nc.dram_tensor(name, shape, dt, kind="ExternalInput").ap()

    din("e_coords", [S, D])
    din("e_id", [S, D])
    din("e_time", [S, D])
    din("mem", [S, D])
    din("saqkvw", [L, D, 3 * D])
    din("saoutw", [L, D, D])
    din("caqkvw", [L, D, 3 * D])
    din("caoutw", [L, D, D])
    din("w1", [L, D, F])
    din("w2", [L, F, D])
    din("mlp1", [D, D])
    din("mlp2", [D, 4])
    din("lngb", [1 + 3 * L, 2, D])
    din("lngcol", [1 + 3 * L, P, DT], F32)
    din("ones128", [P, 1])
    din("onesrow", [1, NQ])
    din("lbias", [L, P, 52], F32)
    din("mlpb1", [P, DT], F32)
    din("mlpb2", [4, 1], F32)
    din("savb", [L, 1, D], F32)
    din("cavb", [L, 1, D], F32)
    din("maskadd", [P, P], F32)
    out_dram = nc.dram_tensor("out", [4, S], F32, kind="ExternalOutput").ap()

    with tile.TileContext(nc) as tc:
        with (
            tc.tile_pool(name="const", bufs=1) as cpool,
            tc.tile_pool(name="w", bufs=6) as wpool,
            tc.tile_pool(name="wf", bufs=3) as wfpool,
            tc.tile_pool(name="bias", bufs=2) as bpool,
            tc.tile_pool(name="act1", bufs=1) as act1,
            tc.tile_pool(name="act2", bufs=2) as act2,
            tc.tile_pool(name="hp", bufs=4) as hpool,
            tc.tile_pool(name="ptq", bufs=6) as ptq,
            tc.tile_pool(name="rows", bufs=4) as rpool,
            tc.tile_pool(name="rows2", bufs=2) as rpool2,
            tc.tile_pool(name="psA", bufs=2, space="PSUM") as psA,
            tc.tile_pool(name="psB", bufs=2, space="PSUM") as psB,
            tc.tile_pool(name="psC", bufs=1, space="PSUM") as psC,
            tc.tile_pool(name="psD", bufs=1, space="PSUM") as psD,
        ):
            # ---- constants ----
            idf = cpool.tile([P, P], F32, tag="idf")
            make_identity(nc, idf)
            identR = cpool.tile([P, P], F32R, tag="idr")
            nc.vector.tensor_copy(identR, idf)
            ones128 = cpool.tile([P, 1], F32R, tag="ones128")
            nc.sync.dma_start(ones128, dr["ones128"])
            mask_sb = cpool.tile([P, P], F32, tag="mask")
            nc.sync.dma_start(mask_sb, dr["maskadd"])
            ident_bf = cpool.tile([P, P], mybir.dt.bfloat16, tag="idb")
            nc.vector.tensor_copy(ident_bf, idf)
            mask_bf = cpool.tile([P, P], mybir.dt.bfloat16, tag="maskb")
            nc.vector.tensor_copy(mask_bf, mask_sb)
            c_eps = cpool.tile([1, 1], F32, tag="ceps")
            nc.vector.memset(c_eps, EPS)
            vcol_f32 = cpool.tile([P, KT * H], F32, tag="vcol")
            nc.vector.memset(vcol_f32, 1.0)
            mlp2w = cpool.tile([P, DT, 4], F32R, tag="mlp2w")
            nc.sync.dma_start(mlp2w, dr["mlp2"].rearrange("(ko p) m -> p ko m", p=P))
            mlpb1 = cpool.tile([P, DT], F32, tag="mlpb1")
            nc.sync.dma_start(mlpb1, dr["mlpb1"])
            mlpb2 = cpool.tile([4, 1], F32, tag="mlpb2")
            nc.sync.dma_start(mlpb2, dr["mlpb2"])
            outsb = cpool.tile([4, S], F32, tag="outsb")
            a2p = cpool.tile([2, NQ], F32R, tag="a2p")
            nc.sync.dma_start(a2p[1:2, :], dr["onesrow"])

            def load_chunks(ap_pdn, col_lo, col_hi, n=DT):
                out = []
                for k in range(n):
                    t = wpool.tile([P, col_hi - col_lo], F32R, tag="proj_k")
                    nc.sync.dma_start(t, ap_pdn[:, k, col_lo:col_hi])
                    out.append(t)
                return out

            def transpose_in(dst, src_seq):
                """src_seq: [P, KT, NQ] seq-major f32r -> dst [P, DT, S] feat-major."""
                for st in range(KT):
                    for j in range(DT):
                        ps_t = psA.tile([P, P], F32R, tag="mm")
                        nc.tensor.matmul(ps_t, src_seq[:, st, sl(j, P)], identR,
                                         is_transpose=True, start=True, stop=True)
                        nc.vector.tensor_copy(dst[:, j, sl(st, P)], ps_t)

            def layer_norm(src, dst, gbi):
                gb = bpool.tile([2, D], F32R, tag="gb")
                nc.sync.dma_start(gb, dr["lngb"][gbi])
                gcol = bpool.tile([P, DT], F32, tag="gcol")
                nc.sync.dma_start(gcol, dr["lngcol"][gbi])
                for i in range(NI):
                    isl = sl(i, NQ)
                    m1 = psC.tile([1, NQ], F32, tag="auxC")
                    m2 = psD.tile([1, NQ], F32, tag="auxD")
                    for j in range(DT):
                        nc.tensor.matmul(m1, ones128, src[:, j, isl],
                                         start=(j == 0), stop=(j == DT - 1))
                    for j in range(DT):
                        sq = hpool.tile([P, NQ], F32R, tag="h")
                        nc.scalar.activation(sq, src[:, j, isl], AF.Square)
                        nc.tensor.matmul(m2, ones128, sq,
                                         start=(j == 0), stop=(j == DT - 1))
                    musq = rpool.tile([1, NQ], F32, tag="r1")
                    nc.scalar.activation(musq, m1, AF.Square)
                    var = rpool.tile([1, NQ], F32, tag="r1")
                    nc.vector.tensor_tensor(var, m2, musq, AX.subtract)
                    lnv = rpool.tile([1, NQ], F32, tag="r1")
                    nc.scalar.activation(lnv, var, AF.Ln, bias=c_eps)
                    rs = rpool.tile([1, NQ], F32, tag="r1")
                    nc.scalar.activation(rs, lnv, AF.Exp, scale=-0.5)
                    nc.vector.scalar_tensor_tensor(a2p[0:1, :], m1, -1.0, rs,
                                                   AX.mult, AX.mult)
                    rs_rep = rpool2.tile([P, NQ], F32, tag="rsrep")
                    nc.gpsimd.partition_broadcast(rs_rep, rs)
                    for j in range(DT):
                        bi = psD.tile([P, NQ], F32, tag="auxD")
                        nc.tensor.matmul(bi, gb[0:2, sl(j, P)], a2p, start=True, stop=True)
                        ht = hpool.tile([P, NQ], F32, tag="h")
                        nc.gpsimd.tensor_tensor(ht, src[:, j, isl], rs_rep, AX.mult)
                        nc.vector.scalar_tensor_tensor(dst[:, j, isl], ht,
                                                       gcol[:, j:j + 1], bi,
                                                       AX.mult, AX.add)

            def attention(i, causal, qTI, kT_, vaug_, oTI):
                t_hi = min(KT, 4 * (i + 1)) if causal else KT
                c0s = [max(0, P * t - NQ * i) if causal else 0
                       for t in range(t_hi)]
                for hp in range(H // 2):
                    hj = hp  # pair (2*hp, 2*hp+1) lives in dtile hp
                    pts = {}
                    for t0 in range(0, t_hi, 2):
                        cp = c0s[t0]  # pair computes from the smaller offset
                        pss = [psA.tile([P, 2, NQ], F32, tag="mm",
                                        name=f"ps{hp}_{u}_{t0}_{i}")
                               for u in range(2)]
                        # interleave the two heads' score matmuls so the
                        # 64-row tiles (rows 0-63 vs 64-127) overlap on PE
                        for u in range(2):
                            t = t0 + u
                            for hu in range(2):
                                nc.tensor.matmul(
                                    pss[hu][:, u, cp:NQ],
                                    kT_[sl(hu, HD), hj, sl(t, P)],
                                    qTI[sl(hu, HD), hj, cp:NQ],
                                    start=True, stop=True)
                            if causal and t >= 4 * i:
                                c0 = c0s[t]
                                for hu in range(2):
                                    nc.tensor.matmul(pss[hu][:, u, c0:c0 + P],
                                                     ident_bf, mask_bf,
                                                     start=False, stop=True,
                                                     skip_group_check=True)
                        for hu in range(2):
                            ptt = ptq.tile([P, 2, NQ], F32R, tag="pt",
                                           name=f"pt{hp}_{hu}_{t0}_{i}")
                            nc.scalar.activation(ptt[:, :, cp:NQ],
                                                 pss[hu][:, :, cp:NQ],
                                                 AF.Exp, scale=0.125)
                            pts[(hu, t0)] = ptt
                    for hu in range(2):
                        h = 2 * hp + hu
                        ps_o = psB.tile([P, NQ], F32, tag="pv",
                                        name=f"pso{hp}_{hu}_{i}")
                        for t in range(t_hi):
                            c0 = c0s[t]
                            nc.tensor.matmul(ps_o[0:HD + 1, c0:NQ],
                                             vaug_[:, t, h, :],
                                             pts[(hu, t - t % 2)][:, t % 2, c0:NQ],
                                             start=(t == 0),
                                             stop=(t == t_hi - 1),
                                             skip_group_check=True)
                        rec = rpool2.tile([1, NQ], F32, tag="rec")
                        nc.vector.reciprocal(rec, ps_o[HD:HD + 1, :])
                        rep = rpool2.tile([HD, NQ], F32, tag="rep")
                        nc.gpsimd.partition_broadcast(rep, rec)
                        nc.vector.tensor_tensor(oTI[sl(hu, HD), hj, :],
                                                ps_o[0:HD, :], rep, AX.mult)

            def proj_feat(chunks, rhs, i, evac, j_tiles=DT):
                isl = sl(i, NQ)
                for j in range(j_tiles):
                    ps = psA.tile([P, NQ], F32, tag="mm")
                    for k in range(DT):
                        nc.tensor.matmul(ps, chunks[k][:, sl(j, P)], rhs[:, k, isl],
                                         start=(k == 0), stop=(k == DT - 1))
                    evac(ps, j, isl)

            def proj_kv(wap, x_src, kT_dst, vaug_dst, lb, qoff, vrep, evac_act):
                """k -> feature-major kT_dst; v -> seq-major into vaug_dst (+bias)."""
                kc = load_chunks(wap, D, 2 * D)
                for i in range(NI):
                    isl = sl(i, NQ)
                    for j in range(DT):
                        ps = psA.tile([P, NQ], F32, tag="mm")
                        for k in range(DT):
                            nc.tensor.matmul(ps, kc[k][:, sl(j, P)], x_src[:, k, isl],
                                             start=(k == 0), stop=(k == DT - 1))
                        nc.vector.tensor_scalar_add(
                            kT_dst[:, j, isl], ps,
                            lb[:, qoff + 4 + j:qoff + 5 + j])
                vc = load_chunks(wap, 2 * D, 3 * D)
                nc.vector.tensor_copy(
                    vaug_dst[:, :, :, HD:HD + 1],
                    vcol_f32.rearrange("p (t h o) -> p t h o", t=KT, h=H))
                for st in range(KT):
                    ps = psA.tile([P, NQ], F32, tag="mm")
                    for k in range(DT):
                        nc.tensor.matmul(ps, x_src[:, k, sl(st, P)], vc[k],
                                         start=(k == 0), stop=(k == DT - 1))
                    nc.vector.tensor_tensor(
                        vaug_dst[:, st, :, 0:HD],
                        ps.rearrange("p (h e) -> p h e", e=HD),
                        vrep.rearrange("p (h e) -> p h e", e=HD),
                        AX.add)

            def attn_block(wap, oap, lb, qoff, ooff, vrow_ap, q_src, kv_src,
                           causal, resid_src, resid_dst):
                vrow = bpool.tile([1, D], F32, tag="vrow")
                nc.sync.dma_start(vrow, vrow_ap)
                vrep = bpool.tile([P, D], F32, tag="vrep")
                nc.gpsimd.partition_broadcast(vrep, vrow)
                kT_ = act1.tile([P, DT, S], F32R, tag="kT")
                vaug_ = act1.tile([P, KT, H, HD + 1], F32R, tag="vaug")
                proj_kv(wap, kv_src, kT_, vaug_, lb, qoff, vrep, causal)
                for i in range(NI):
                    isl = sl(i, NQ)
                    qc = load_chunks(wap, 0, D)
                    oc = load_chunks(oap, 0, D)
                    qTI = act1.tile([P, DT, NQ], F32R, tag="qT")

                    def evq(ps, j, _isl, qTI=qTI):
                        nc.vector.tensor_scalar_add(
                            qTI[:, j, :], ps, lb[:, qoff + j:qoff + j + 1])
                    proj_feat(qc, q_src, i, evq)
                    oTI = act1.tile([P, DT, NQ], F32R, tag="oT")
                    attention(i, causal, qTI, kT_, vaug_, oTI)
                    for j in range(DT):
                        ps = psA.tile([P, NQ], F32, tag="mm")
                        for k in range(DT):
                            nc.tensor.matmul(ps, oc[k][:, sl(j, P)], oTI[:, k, :],
                                             start=(k == 0), stop=(k == DT - 1))
                        nc.vector.scalar_tensor_tensor(
                            resid_dst[:, j, isl], ps, lb[:, ooff + j:ooff + j + 1],
                            resid_src[:, j, isl], AX.add, AX.add)

            # ================= embedding =================
            e1 = act1.tile([P, KT, NQ], F32R, tag="kT")
            e2 = act1.tile([P, KT, NQ], F32R, tag="memT")
            e3 = act1.tile([P, KT, NQ], F32R, tag="vaug")
            nc.sync.dma_start(e1, dr["e_coords"].rearrange("(t p) d -> p t d", p=P))
            nc.sync.dma_start(e2, dr["e_id"].rearrange("(t p) d -> p t d", p=P))
            nc.sync.dma_start(e3, dr["e_time"].rearrange("(t p) d -> p t d", p=P))
            s1 = act2.tile([P, KT, NQ], F32R, tag="xT")
            est = act2.tile([P, KT, NQ], F32R, tag="xT")
            for st in range(KT):
                nc.vector.tensor_tensor(s1[:, st], e1[:, st], e2[:, st], AX.add)
                nc.vector.tensor_tensor(est[:, st], s1[:, st], e3[:, st], AX.add)
            resid = act2.tile([P, DT, S], F32R, tag="xT")
            transpose_in(resid, est)
            x = act2.tile([P, DT, S], F32R, tag="xT")
            layer_norm(resid, x, 0)

            mseq = act2.tile([P, KT, NQ], F32R, tag="xT")
            nc.sync.dma_start(mseq, dr["mem"].rearrange("(t p) d -> p t d", p=P))
            memT = act1.tile([P, DT, S], F32R, tag="memT")
            transpose_in(memT, mseq)

            # ================= layers =================
            for l in range(L):
                saqkvw = dr["saqkvw"][l].rearrange("(ko p) n -> p ko n", p=P)
                saoutw = dr["saoutw"][l].rearrange("(ko p) n -> p ko n", p=P)
                caqkvw = dr["caqkvw"][l].rearrange("(ko p) n -> p ko n", p=P)
                caoutw = dr["caoutw"][l].rearrange("(ko p) n -> p ko n", p=P)

                lb = bpool.tile([P, 52], F32, tag="lb")
                nc.sync.dma_start(lb, dr["lbias"][l])

                resid = act2.tile([P, DT, S], F32R, tag="xT")
                attn_block(saqkvw, saoutw, lb, 0, 12, dr["savb"][l],
                           x, x, True, x, resid)
                x = act2.tile([P, DT, S], F32R, tag="xT")
                layer_norm(resid, x, 1 + 3 * l)

                resid = act2.tile([P, DT, S], F32R, tag="xT")
                attn_block(caqkvw, caoutw, lb, 16, 28, dr["cavb"][l],
                           x, memT, False, x, resid)
                x = act2.tile([P, DT, S], F32R, tag="xT")
                layer_norm(resid, x, 2 + 3 * l)

                # ---- FFN ----
                w1r = dr["w1"][l].rearrange("(ko p) f -> p ko f", p=P)
                w2r = dr["w2"][l].rearrange("(fo p) n -> p fo n", p=P)
                resid = act2.tile([P, DT, S], F32R, tag="xT")
                for i in range(NI):
                    isl = sl(i, NQ)
                    ys = [psB.tile([P, NQ], F32, tag="pv", name=f"y0_{l}_{i}"),
                          psB.tile([P, NQ], F32, tag="pv", name=f"y1_{l}_{i}"),
                          psC.tile([P, NQ], F32, tag="auxC", name=f"y2_{l}_{i}"),
                          psD.tile([P, NQ], F32, tag="auxD", name=f"y3_{l}_{i}")]
                    for fp in range(FT // 2):
                        w1f = wfpool.tile([P, DT, 2 * P], F32R, tag="w1f")
                        nc.sync.dma_start(w1f, w1r[:, :, sl(fp, 2 * P)])
                        w2f = wfpool.tile([P, 2, D], F32R, tag="w2f")
                        nc.sync.dma_start(w2f, w2r[:, 2 * fp:2 * fp + 2, :])
                        for fi in range(2):
                            f = 2 * fp + fi
                            ps_h = psA.tile([P, NQ], F32, tag="mm")
                            for k in range(DT):
                                nc.tensor.matmul(ps_h, w1f[:, k, sl(fi, P)],
                                                 x[:, k, isl],
                                                 start=(k == 0), stop=(k == DT - 1))
                            hT = hpool.tile([P, NQ], F32R, tag="h")
                            nc.scalar.activation(hT, ps_h, AF.Relu,
                                                 bias=lb[:, 32 + f:33 + f])
                            for j in range(DT):
                                nc.tensor.matmul(ys[j], w2f[:, fi, sl(j, P)], hT,
                                                 start=(f == 0), stop=(f == FT - 1))
                    for j in range(DT):
                        nc.vector.scalar_tensor_tensor(
                            resid[:, j, isl], ys[j], lb[:, 48 + j:49 + j],
                            x[:, j, isl], AX.add, AX.add)
                x = act2.tile([P, DT, S], F32R, tag="xT")
                layer_norm(resid, x, 3 + 3 * l)

            # ================= final MLP =================
            m1c = load_chunks(dr["mlp1"].rearrange("(ko p) n -> p ko n", p=P), 0, D)
            for i in range(NI):
                isl = sl(i, NQ)
                hmI = act1.tile([P, DT, NQ], F32R, tag="qT")

                def evh(ps, j, _isl, hmI=hmI):
                    nc.scalar.activation(hmI[:, j, :], ps, AF.Relu,
                                         bias=mlpb1[:, j:j + 1])
                proj_feat(m1c, x, i, evh)
                ps_p = psC.tile([4, NQ], F32, tag="auxC")
                for k in range(DT):
                    nc.tensor.matmul(ps_p, mlp2w[:, k, :], hmI[:, k, :],
                                     start=(k == 0), stop=(k == DT - 1))
                nc.vector.tensor_scalar_add(outsb[:, isl], ps_p, mlpb2)
            nc.sync.dma_start(out_dram, outsb)

    nc.compile()
    return nc


_NC = None


def _get_nc():
    global _NC
    if _NC is None:
        _NC = build_nc()
    return _NC


def _prep_in_maps(inputs):
    f32 = np.float32

    def tr(w):  # [..., O, I] -> [..., I, O]
        return np.ascontiguousarray(np.swapaxes(np.asarray(w, f32), -1, -2))

    def packb(b, ntiles):  # [L, ntiles*P] -> [L, P, ntiles]
        b = np.asarray(b, f32)
        return np.ascontiguousarray(b.reshape(L, ntiles, P).transpose(0, 2, 1))

    lngb = np.zeros((1 + 3 * L, 2, D), f32)
    lngb[0, 0] = np.asarray(inputs["ln_emb_g"], f32)
    lngb[0, 1] = np.asarray(inputs["ln_emb_b"], f32)
    for l in range(L):
        lngb[1 + 3 * l, 0] = np.asarray(inputs["ln1_g"][l], f32)
        lngb[1 + 3 * l, 1] = np.asarray(inputs["ln1_b"][l], f32)
        lngb[2 + 3 * l, 0] = np.asarray(inputs["ln2_g"][l], f32)
        lngb[2 + 3 * l, 1] = np.asarray(inputs["ln2_b"][l], f32)
        lngb[3 + 3 * l, 0] = np.asarray(inputs["ln3_g"][l], f32)
        lngb[3 + 3 * l, 1] = np.asarray(inputs["ln3_b"][l], f32)

    idx = np.arange(P) // A
    maskadd = np.where(idx[:, None] <= idx[None, :], 0.0, -1e9).astype(f32)

    shared = {
        "saqkvw": tr(inputs["sa_qkv_w"]),
        "saoutw": tr(inputs["sa_out_w"]),
        "caqkvw": tr(inputs["ca_qkv_w"]),
        "caoutw": tr(inputs["ca_out_w"]),
        "w1": tr(inputs["ffn_w1"]),
        "w2": tr(inputs["ffn_w2"]),
        "mlp1": tr(inputs["mlp_w1"]),
        "mlp2": tr(inputs["mlp_w2"]),
        "lngb": lngb,
        "lngcol": np.ascontiguousarray(
            lngb[:, 0].reshape(1 + 3 * L, DT, P).transpose(0, 2, 1)),
        "ones128": np.full((P, 1), 1.0 / D, f32),
        "onesrow": np.ones((1, NQ), f32),
        "lbias": np.concatenate([
            packb(inputs["sa_qkv_b"], 12),
            packb(inputs["sa_out_b"], DT),
            packb(inputs["ca_qkv_b"], 12),
            packb(inputs["ca_out_b"], DT),
            packb(inputs["ffn_b1"], FT),
            packb(inputs["ffn_b2"], DT),
        ], axis=2),
        "mlpb1": np.ascontiguousarray(
            np.asarray(inputs["mlp_b1"], f32).reshape(DT, P).T),
        "mlpb2": np.asarray(inputs["mlp_b2"], f32).reshape(4, 1),
        "savb": np.ascontiguousarray(
            np.asarray(inputs["sa_qkv_b"], f32)[:, 2 * D:3 * D].reshape(L, 1, D)),
        "cavb": np.ascontiguousarray(
            np.asarray(inputs["ca_qkv_b"], f32)[:, 2 * D:3 * D].reshape(L, 1, D)),
        "maskadd": maskadd,
    }

    coords = np.asarray(inputs["coords_embeddings"], f32).reshape(B, S, D)
    ids = np.asarray(inputs["id_embeddings"], f32).reshape(B, S, D)
    times = np.asarray(inputs["timestep_embeddings"], f32).reshape(B, S, D)
    mem = np.asarray(inputs["encoder_embeddings"], f32)

    in_maps = []
    for b in range(B):
        m = dict(shared)
        m["e_coords"] = np.ascontiguousarray(coords[b])
        m["e_id"] = np.ascontiguousarray(ids[b])
        m["e_time"] = np.ascontiguousarray(times[b])
        m["mem"] = np.ascontiguousarray(mem[b])
        in_maps.append(m)
    return in_maps


def kernel(**inputs):
    nc = _get_nc()
    in_maps = _prep_in_maps(inputs)
    res = run_bass_kernel_spmd(nc, in_maps, core_ids=list(range(B)))
    outs = []
    for b in range(B):
        o = res.results[b]["out"]          # [4, S]
        outs.append(o.T.reshape(T, A, 4, 1))
    return np.stack(outs).astype(np.float32)
